# revision 3
# baseline (speedup 1.0000x reference)
"""Multi-head attention (B=1, L=2048, D=1024, H=16) on 8 TRN2 NeuronCores.

Sharding: tensor-parallel over heads. Core i computes heads 2i, 2i+1:
  - projections with column shards of w_q/w_k/w_v (128 cols each)
  - full attention for its 2 heads
  - partial output projection with the matching 128-row shard of w_o
Host sums the 8 partial outputs (row-split w_concat => partial-sum combine).

Layout strategy on-device (all matmuls bf16, fp32 PSUM accumulate):
  - host ships q^T/k^T/v^T so the contraction dim (D) lands on SBUF partitions
  - projections produce qh^T/kh^T [128=2*dh, L] directly (lhsT = weight shard)
  - scores computed transposed: S^T[k, q] = kh^T^T... i.e. lhsT=kh^T slice,
    rhs=qh^T -> psum [128 kseq, 1024 qseq]; exp on ScalarE (scale=1/8 folded)
  - P~ @ V via lhsT = [vh | ones] (ones column makes row 64 of the psum the
    softmax denominator), accumulated over kseq tiles
  - normalize with VectorE reciprocal + a K=1 broadcast matmul
  - out_partial = concat_local^T^T... lhsT = normalized concat^T [128, L]
"""

import os
import numpy as np
import ml_dtypes

import concourse.bass as bass
import concourse.mybir as mybir
import concourse.tile as tile
from concourse import bacc
from concourse.bass import ts
from concourse.bass_utils import run_bass_kernel_spmd
from concourse.masks import make_identity

P = 128
L = 2048
D = 1024
DH = 64
NCORES = 8
BF16 = mybir.dt.bfloat16
F32 = mybir.dt.float32
AF = mybir.ActivationFunctionType
ALU = mybir.AluOpType

TRACE = False  # test.py flips this to get an NTFF profile / exec_time_ns
LAST_RESULT = {}

_CACHED_NC = None


def _build():
    nc = bacc.Bacc("TRN2", target_bir_lowering=False, debug=False, num_devices=NCORES)

    qT = nc.dram_tensor("qT", [D, L], BF16, kind="ExternalInput")
    kT = nc.dram_tensor("kT", [D, L], BF16, kind="ExternalInput")
    vT = nc.dram_tensor("vT", [D, L], BF16, kind="ExternalInput")
    wq = nc.dram_tensor("wq", [D, P], BF16, kind="ExternalInput")
    wk = nc.dram_tensor("wk", [D, P], BF16, kind="ExternalInput")
    wv = nc.dram_tensor("wv", [D, P], BF16, kind="ExternalInput")
    bq = nc.dram_tensor("bq", [P, 1], F32, kind="ExternalInput")
    bk = nc.dram_tensor("bk", [P, 1], F32, kind="ExternalInput")
    bv = nc.dram_tensor("bv", [P, 1], F32, kind="ExternalInput")
    wo = nc.dram_tensor("wo", [P, D], BF16, kind="ExternalInput")
    bo = nc.dram_tensor("bo", [P, D], F32, kind="ExternalInput")
    out = nc.dram_tensor("out", [L, D], F32, kind="ExternalOutput")

    KT = D // P  # 8 contraction tiles for the projections
    LT = L // P  # 16 seq tiles

    with tile.TileContext(nc) as tc:
        with (
            tc.tile_pool(name="const", bufs=1) as const_pool,
            tc.tile_pool(name="inputs", bufs=1) as in_pool,
            tc.tile_pool(name="proj", bufs=1) as proj_pool,
            tc.tile_pool(name="work", bufs=1) as work_pool,
        ):
            identity = const_pool.tile([P, P], BF16)
            make_identity(nc, identity[:])
            ones64 = const_pool.tile([65, DH], F32)
            nc.vector.memset(ones64[:], 1.0)

            # ---- stage inputs ----
            qT_sb = in_pool.tile([P, KT, L], BF16)
            kT_sb = in_pool.tile([P, KT, L], BF16)
            vT_sb = in_pool.tile([P, KT, L], BF16)
            for dst, src in ((qT_sb, qT), (kT_sb, kT), (vT_sb, vT)):
                src_r = src.rearrange("(t p) l -> p t l", p=P)
                for t in range(KT):
                    nc.sync.dma_start(dst[:, t, :], src_r[:, t, :])
            wq_sb = in_pool.tile([P, KT, P], BF16)
            wk_sb = in_pool.tile([P, KT, P], BF16)
            wv_sb = in_pool.tile([P, KT, P], BF16)
            for dst, src in ((wq_sb, wq), (wk_sb, wk), (wv_sb, wv)):
                nc.sync.dma_start(dst[:], src.rearrange("(t p) m -> p t m", p=P))
            bq_sb = in_pool.tile([P, 1], F32)
            bk_sb = in_pool.tile([P, 1], F32)
            bv_sb = in_pool.tile([P, 1], F32)
            for dst, src in ((bq_sb, bq), (bk_sb, bk), (bv_sb, bv)):
                nc.sync.dma_start(dst[:], src[:])
            wo_sb = in_pool.tile([P, D], BF16)
            nc.sync.dma_start(wo_sb[:], wo[:])
            bo_sb = in_pool.tile([P, D], F32)
            nc.sync.dma_start(bo_sb[:], bo[:])

            # ---- projections: qh^T / kh^T / vh^T  [128 (2 heads * 64), L] ----
            qhT = proj_pool.tile([P, L], BF16)
            khT = proj_pool.tile([P, L], BF16)
            vhT = proj_pool.tile([P, L], BF16)
            with tc.tile_pool(name="pp1", bufs=2, space="PSUM") as pp1:
                for w_sb, b_sb, x_sb, dst in (
                    (wq_sb, bq_sb, qT_sb, qhT),
                    (wk_sb, bk_sb, kT_sb, khT),
                    (wv_sb, bv_sb, vT_sb, vhT),
                ):
                    for n in range(L // 512):
                        ps = pp1.tile([P, 512], F32, tag="projps")
                        for t in range(KT):
                            nc.tensor.matmul(
                                ps[:],
                                w_sb[:, t, :],
                                x_sb[:, t, ts(n, 512)],
                                start=(t == 0),
                                stop=(t == KT - 1),
                            )
                        nc.vector.tensor_scalar(
                            dst[:, ts(n, 512)], ps[:], b_sb[:], None, op0=ALU.add
                        )

                # vh natural layout [kseq, 130]: cols 0:64 head A, 64 ones,
                # 65:129 head B, 129 ones  (ones column = softmax denominator)
                vh_sb = proj_pool.tile([P, LT, 130], BF16)
                nc.vector.memset(vh_sb[:], 1.0)
                for t2 in range(LT):
                    pst = pp1.tile([P, P], BF16, tag="trps")
                    nc.tensor.transpose(pst[:], vhT[:, ts(t2, P)], identity[:])
                    nc.vector.tensor_copy(vh_sb[:, t2, 0:DH], pst[:, 0:DH])
                    nc.vector.tensor_copy(vh_sb[:, t2, 65 : 65 + DH], pst[:, DH:P])

            # ---- attention (heads sequential, qseq in halves of 1024) ----
            lhsT_c = work_pool.tile([P, L], BF16)  # normalized concat^T
            uA = work_pool.tile([DH, L], F32)
            uB = work_pool.tile([DH, L], F32)
            with (
                tc.tile_pool(name="att_ps", bufs=1, space="PSUM") as att_ps,
                tc.tile_pool(name="st_ps", bufs=2, space="PSUM") as st_ps,
                tc.tile_pool(name="pt_pool", bufs=3) as pt_pool,
                tc.tile_pool(name="fin_pool", bufs=2) as fin_pool,
            ):
                for h in (0, 1):
                    u_dst = uA if h == 0 else uB
                    for qh in (0, 1):
                        av = att_ps.tile([65, 1024], F32, tag="av")
                        for kt in range(LT):
                            st = st_ps.tile([P, 1024], F32, tag="st")
                            for j in (0, 1):
                                nc.tensor.matmul(
                                    st[:, ts(j, 512)],
                                    khT[ts(h, DH), ts(kt, P)],
                                    qhT[ts(h, DH), qh * 1024 + j * 512 : qh * 1024 + (j + 1) * 512],
                                )
                            pt = pt_pool.tile([P, 1024], BF16, tag="pt")
                            nc.scalar.activation(pt[:], st[:], AF.Exp, scale=0.125)
                            for j in (0, 1):
                                nc.tensor.matmul(
                                    av[:, ts(j, 512)],
                                    vh_sb[:, kt, 65 * h : 65 * h + 65],
                                    pt[:, ts(j, 512)],
                                    start=(kt == 0),
                                    stop=(kt == LT - 1),
                                )
                        # evict: rows 0-63 = unnormalized out^T, row 64 = denom
                        nc.vector.tensor_copy(u_dst[:, ts(qh, 1024)], av[0:DH, :])
                        dt_ = fin_pool.tile([65, 1024], F32, tag="dt")
                        nc.vector.reciprocal(dt_[64:65, :], av[64:65, :])
                        # broadcast recip across 64 partitions via K=1 matmul
                        bc = att_ps.tile([DH, 1024], F32, tag="bc")
                        for j in (0, 1):
                            nc.tensor.matmul(
                                bc[:, ts(j, 512)],
                                ones64[64:65, :],
                                dt_[64:65, ts(j, 512)],
                            )
                        if h == 0:
                            nc.vector.tensor_tensor(
                                lhsT_c[0:DH, ts(qh, 1024)],
                                u_dst[:, ts(qh, 1024)],
                                bc[:],
                                op=ALU.mult,
                            )
                        else:
                            nb = fin_pool.tile([DH, 1024], BF16, tag="nb")
                            nc.vector.tensor_tensor(
                                nb[:], u_dst[:, ts(qh, 1024)], bc[:], op=ALU.mult
                            )
                            # partition shift 0-63 -> 64-127 via SBUF->SBUF DMA
                            nc.sync.dma_start(lhsT_c[DH:P, ts(qh, 1024)], nb[:])

            # ---- output projection: out_partial = concat_local @ wo_shard ----
            with (
                tc.tile_pool(name="op_ps", bufs=2, space="PSUM") as op_ps,
                tc.tile_pool(name="out_pool", bufs=2) as out_pool,
            ):
                for m in range(LT):
                    osb = out_pool.tile([P, D], F32, tag="osb")
                    for n in (0, 1):
                        ps = op_ps.tile([P, 512], F32, tag="ops")
                        nc.tensor.matmul(
                            ps[:], lhsT_c[:, ts(m, P)], wo_sb[:, ts(n, 512)]
                        )
                        nc.vector.tensor_tensor(
                            osb[:, ts(n, 512)], ps[:], bo_sb[:, ts(n, 512)], op=ALU.add
                        )
                    nc.sync.dma_start(out[ts(m, P), :], osb[:])

    nc.compile()
    return nc


def kernel(q, k, v, w_q, b_q, w_k, b_k, w_v, b_v, w_o, b_o):
    global _CACHED_NC, LAST_RESULT
    if _CACHED_NC is None:
        _CACHED_NC = _build()
    nc = _CACHED_NC

    bf16 = ml_dtypes.bfloat16
    q2 = np.ascontiguousarray(np.asarray(q, np.float32)[0].T).astype(bf16)
    k2 = np.ascontiguousarray(np.asarray(k, np.float32)[0].T).astype(bf16)
    v2 = np.ascontiguousarray(np.asarray(v, np.float32)[0].T).astype(bf16)
    w_q = np.asarray(w_q, np.float32)
    w_k = np.asarray(w_k, np.float32)
    w_v = np.asarray(w_v, np.float32)
    w_o = np.asarray(w_o, np.float32)
    b_q = np.asarray(b_q, np.float32)
    b_k = np.asarray(b_k, np.float32)
    b_v = np.asarray(b_v, np.float32)
    b_o = np.asarray(b_o, np.float32)

    in_maps = []
    for i in range(NCORES):
        sl = slice(P * i, P * (i + 1))
        bo_i = (
            np.ascontiguousarray(np.broadcast_to(b_o, (P, D))).astype(np.float32)
            if i == 0
            else np.zeros((P, D), np.float32)
        )
        in_maps.append(
            {
                "qT": q2,
                "kT": k2,
                "vT": v2,
                "wq": np.ascontiguousarray(w_q[:, sl]).astype(bf16),
                "wk": np.ascontiguousarray(w_k[:, sl]).astype(bf16),
                "wv": np.ascontiguousarray(w_v[:, sl]).astype(bf16),
                "bq": np.ascontiguousarray(b_q[sl]).reshape(P, 1),
                "bk": np.ascontiguousarray(b_k[sl]).reshape(P, 1),
                "bv": np.ascontiguousarray(b_v[sl]).reshape(P, 1),
                "wo": np.ascontiguousarray(w_o[sl, :]).astype(bf16),
                "bo": bo_i,
            }
        )

    kwargs = {}
    if TRACE:
        tdir = "/tmp/bass_trace"
        os.makedirs(tdir, exist_ok=True)
        kwargs["tmpdir"] = tdir
    res = run_bass_kernel_spmd(nc, in_maps, list(range(NCORES)), trace=TRACE, **kwargs)
    LAST_RESULT = {
        "exec_time_ns": res.exec_time_ns,
        "trace_path": (res.instructions_and_trace or (None, None))[1],
    }
    acc = np.zeros((L, D), np.float64)
    for i in range(NCORES):
        acc += res.results[i]["out"].astype(np.float64)
    return acc.astype(np.float32).reshape(1, L, D)


# revision 6
# speedup vs baseline: 1.0805x; 1.0805x over previous
"""Multi-head attention (B=1, L=2048, D=1024, H=16) on 8 TRN2 NeuronCores.

Sharding: tensor-parallel over heads. Core i computes heads 2i, 2i+1:
  - projections with column shards of w_q/w_k/w_v (128 cols each)
  - full attention for its 2 heads
  - partial output projection with the matching 128-row shard of w_o
Host sums the 8 partial outputs (row-split w_concat => partial-sum combine).

Layout strategy on-device (all matmuls bf16, fp32 PSUM accumulate):
  - host ships q^T/k^T/v^T so the contraction dim (D) lands on SBUF partitions
  - projections produce qh^T/kh^T [128=2*dh, L] directly (lhsT = weight shard)
  - scores computed transposed: S^T[k, q]: lhsT=kh^T slice, rhs=qh^T
    -> psum [128 kseq, 1024 qseq]; heads A/B run in different PE row groups
    (base partitions 0/64) so their K=64 matmuls pack; exp on ScalarE
    (scale=1/8 folded into the activation)
  - P~ @ V via lhsT = [vh | ones] (ones column makes row 64 of the psum the
    softmax denominator), accumulated over kseq tiles
  - normalization deferred to after attention: reciprocal computed
    partition-parallel (SBUF->SBUF DMA spread), broadcast via K=1 matmul
  - out_partial (bf16) = concat_local @ wo_shard, host sums in fp32
"""

import os
import numpy as np
import ml_dtypes

import concourse.bass as bass
import concourse.mybir as mybir
import concourse.tile as tile
from concourse import bacc
from concourse.bass import ts
from concourse.bass_utils import run_bass_kernel_spmd
from concourse.masks import make_identity

P = 128
L = 2048
D = 1024
DH = 64
NCORES = 8
BF16 = mybir.dt.bfloat16
F32 = mybir.dt.float32
AF = mybir.ActivationFunctionType
ALU = mybir.AluOpType

TRACE = False  # test.py flips this to get an NTFF profile / exec_time_ns
LAST_RESULT = {}

_CACHED_NC = None


def _build():
    nc = bacc.Bacc("TRN2", target_bir_lowering=False, debug=False, num_devices=NCORES)

    qT = nc.dram_tensor("qT", [D, L], BF16, kind="ExternalInput")
    kT = nc.dram_tensor("kT", [D, L], BF16, kind="ExternalInput")
    vT = nc.dram_tensor("vT", [D, L], BF16, kind="ExternalInput")
    wq = nc.dram_tensor("wq", [D, P], BF16, kind="ExternalInput")
    wk = nc.dram_tensor("wk", [D, P], BF16, kind="ExternalInput")
    wv = nc.dram_tensor("wv", [D, P], BF16, kind="ExternalInput")
    bq = nc.dram_tensor("bq", [P, 1], F32, kind="ExternalInput")
    bk = nc.dram_tensor("bk", [P, 1], F32, kind="ExternalInput")
    bv = nc.dram_tensor("bv", [P, 1], F32, kind="ExternalInput")
    wo = nc.dram_tensor("wo", [P, D], BF16, kind="ExternalInput")
    bo = nc.dram_tensor("bo", [P, D], F32, kind="ExternalInput")
    out = nc.dram_tensor("out", [L, D], BF16, kind="ExternalOutput")

    KT = D // P  # 8 contraction tiles for the projections
    LT = L // P  # 16 seq tiles

    with tile.TileContext(nc) as tc:
        with (
            tc.tile_pool(name="const", bufs=1) as const_pool,
            tc.tile_pool(name="inputs", bufs=1) as in_pool,
            tc.tile_pool(name="proj", bufs=1) as proj_pool,
            tc.tile_pool(name="work", bufs=1) as work_pool,
        ):
            identity = const_pool.tile([P, P], BF16)
            make_identity(nc, identity[:])
            ones64 = const_pool.tile([65, DH], F32)
            nc.vector.memset(ones64[:], 1.0)

            # ---- stage inputs; round-robin DMA queues so loads parallelize.
            # q first (projection q starts as soon as it lands), then k, v.
            qeng = [nc.sync, nc.scalar, nc.gpsimd]
            _rr = [0]

            def dma(dst_ap, src_ap):
                qeng[_rr[0] % len(qeng)].dma_start(dst_ap, src_ap)
                _rr[0] += 1

            wq_sb = in_pool.tile([P, KT, P], BF16)
            wk_sb = in_pool.tile([P, KT, P], BF16)
            wv_sb = in_pool.tile([P, KT, P], BF16)
            for dst, src in ((wq_sb, wq), (wk_sb, wk), (wv_sb, wv)):
                dma(dst[:], src.rearrange("(t p) m -> p t m", p=P))
            bq_sb = in_pool.tile([P, 1], F32)
            bk_sb = in_pool.tile([P, 1], F32)
            bv_sb = in_pool.tile([P, 1], F32)
            for dst, src in ((bq_sb, bq), (bk_sb, bk), (bv_sb, bv)):
                dma(dst[:], src[:])
            wo_sb = in_pool.tile([P, D], BF16)
            dma(wo_sb[:], wo[:])
            bo_sb = in_pool.tile([P, D], F32)
            dma(bo_sb[:], bo[:])

            qT_sb = in_pool.tile([P, KT, L], BF16)
            kT_sb = in_pool.tile([P, KT, L], BF16)
            vT_sb = in_pool.tile([P, KT, L], BF16)
            for dst, src in ((qT_sb, qT), (kT_sb, kT), (vT_sb, vT)):
                src_r = src.rearrange("(t p) l -> p t l", p=P)
                for t in range(KT):
                    for half in range(2):
                        dma(dst[:, t, ts(half, L // 2)], src_r[:, t, ts(half, L // 2)])

            # ---- projections: qh^T / kh^T / vh^T  [128 (2 heads * 64), L] ----
            qhT = proj_pool.tile([P, L], BF16)
            khT = proj_pool.tile([P, L], BF16)
            vhT = proj_pool.tile([P, L], BF16)
            with tc.tile_pool(name="pp1", bufs=2, space="PSUM") as pp1:
                for w_sb, b_sb, x_sb, dst in (
                    (wq_sb, bq_sb, qT_sb, qhT),
                    (wk_sb, bk_sb, kT_sb, khT),
                    (wv_sb, bv_sb, vT_sb, vhT),
                ):
                    for n in range(L // 512):
                        ps = pp1.tile([P, 512], F32, tag="projps")
                        for t in range(KT):
                            nc.tensor.matmul(
                                ps[:],
                                w_sb[:, t, :],
                                x_sb[:, t, ts(n, 512)],
                                start=(t == 0),
                                stop=(t == KT - 1),
                            )
                        nc.vector.tensor_scalar(
                            dst[:, ts(n, 512)], ps[:], b_sb[:], None, op0=ALU.add
                        )

                # vh natural layout [kseq, 130]: cols 0:64 head A, col 64 ones,
                # 65:129 head B, col 129 ones (ones col = softmax denominator)
                vh_sb = proj_pool.tile([P, LT, 130], BF16)
                nc.vector.memset(vh_sb[:], 1.0)
                for t2 in range(LT):
                    pst = pp1.tile([P, P], BF16, tag="projps")
                    nc.tensor.transpose(pst[:], vhT[:, ts(t2, P)], identity[:])
                    nc.vector.tensor_copy(vh_sb[:, t2, 0:DH], pst[:, 0:DH])
                    nc.vector.tensor_copy(vh_sb[:, t2, 65 : 65 + DH], pst[:, DH:P])

            # ---- attention: both heads interleaved (PE row groups 0 / 64),
            # qseq processed in halves of 1024 ----
            lhsT_c = work_pool.tile([P, L], BF16)  # normalized concat^T
            uA = work_pool.tile([DH, L], F32)
            uB = work_pool.tile([DH, L], F32)
            dall = work_pool.tile([65, 2, L], F32)  # row 64 = denominators
            with (
                tc.tile_pool(name="att_ps", bufs=1, space="PSUM") as att_ps,
                tc.tile_pool(name="pt_pool", bufs=2) as pt_pool,
            ):
                for qh in (0, 1):
                    avs = {}
                    for h in (0, 1):
                        av_t = att_ps.tile([65, 1024], F32, tag=f"av{h}", name=f"av{h}_{qh}")
                        avs[h] = av_t
                    for kt in range(LT):
                        pts = {}
                        for h in (0, 1):
                            st = att_ps.tile([P, 1024], F32, tag=f"st{h}")
                            for j in (0, 1):
                                nc.tensor.matmul(
                                    st[:, ts(j, 512)],
                                    khT[ts(h, DH), ts(kt, P)],
                                    qhT[ts(h, DH), qh * 1024 + j * 512 : qh * 1024 + (j + 1) * 512],
                                )
                            pt = pt_pool.tile([P, 1024], BF16, tag=f"pt{h}")
                            nc.scalar.activation(pt[:], st[:], AF.Exp, scale=0.125)
                            pts[h] = pt
                        for h in (0, 1):
                            for j in (0, 1):
                                nc.tensor.matmul(
                                    avs[h][:, ts(j, 512)],
                                    vh_sb[:, kt, 65 * h : 65 * h + 65],
                                    pts[h][:, ts(j, 512)],
                                    start=(kt == 0),
                                    stop=(kt == LT - 1),
                                )
                    for h in (0, 1):
                        u_dst = uA if h == 0 else uB
                        nc.vector.tensor_copy(u_dst[:, ts(qh, 1024)], avs[h][0:DH, :])
                        nc.vector.tensor_copy(
                            dall[64:65, h, ts(qh, 1024)], avs[h][64:65, :]
                        )

            # ---- finalize: partition-parallel reciprocal, broadcast, scale ----
            dsp = work_pool.tile([P, 32], F32)
            nb = work_pool.tile([DH, L], BF16)
            # spread 4096 denominators across partitions, invert, put back
            nc.sync.dma_start(dsp[:], dall[64:65, :, :].rearrange("a h q -> a (h q)"))
            nc.vector.reciprocal(dsp[:], dsp[:])
            nc.sync.dma_start(dall[64:65, :, :].rearrange("a h q -> a (h q)"), dsp[:])
            with tc.tile_pool(name="fin_ps", bufs=2, space="PSUM") as fin_ps:
                for h in (0, 1):
                    bc = fin_ps.tile([DH, L], F32, tag="bc")
                    for j2 in range(L // 512):
                        nc.tensor.matmul(
                            bc[:, ts(j2, 512)],
                            ones64[64:65, :],
                            dall[64:65, h, ts(j2, 512)],
                        )
                    if h == 0:
                        nc.vector.tensor_tensor(
                            lhsT_c[0:DH, :], uA[:], bc[:], op=ALU.mult
                        )
                    else:
                        nc.vector.tensor_tensor(nb[:], uB[:], bc[:], op=ALU.mult)
                        # partition shift 0-63 -> 64-127 via SBUF->SBUF DMA
                        nc.gpsimd.dma_start(lhsT_c[DH:P, :], nb[:])

            # ---- output projection: out_partial = concat_local @ wo_shard ----
            with (
                tc.tile_pool(name="op_ps", bufs=4, space="PSUM") as op_ps,
                tc.tile_pool(name="out_pool", bufs=3) as out_pool,
            ):
                for m in range(LT):
                    osb = out_pool.tile([P, D], BF16, tag="osb")
                    for n in (0, 1):
                        ps = op_ps.tile([P, 512], F32, tag="ops")
                        nc.tensor.matmul(
                            ps[:], lhsT_c[:, ts(m, P)], wo_sb[:, ts(n, 512)]
                        )
                        nc.vector.tensor_tensor(
                            osb[:, ts(n, 512)], ps[:], bo_sb[:, ts(n, 512)], op=ALU.add
                        )
                    (nc.sync if m % 2 == 0 else nc.gpsimd).dma_start(
                        out[ts(m, P), :], osb[:]
                    )

    nc.compile()
    return nc


def kernel(q, k, v, w_q, b_q, w_k, b_k, w_v, b_v, w_o, b_o):
    global _CACHED_NC, LAST_RESULT
    if _CACHED_NC is None:
        _CACHED_NC = _build()
    nc = _CACHED_NC

    bf16 = ml_dtypes.bfloat16
    q2 = np.ascontiguousarray(np.asarray(q, np.float32)[0].T).astype(bf16)
    k2 = np.ascontiguousarray(np.asarray(k, np.float32)[0].T).astype(bf16)
    v2 = np.ascontiguousarray(np.asarray(v, np.float32)[0].T).astype(bf16)
    w_q = np.asarray(w_q, np.float32)
    w_k = np.asarray(w_k, np.float32)
    w_v = np.asarray(w_v, np.float32)
    w_o = np.asarray(w_o, np.float32)
    b_q = np.asarray(b_q, np.float32)
    b_k = np.asarray(b_k, np.float32)
    b_v = np.asarray(b_v, np.float32)
    b_o = np.asarray(b_o, np.float32)

    in_maps = []
    for i in range(NCORES):
        sl = slice(P * i, P * (i + 1))
        bo_i = (
            np.ascontiguousarray(np.broadcast_to(b_o, (P, D))).astype(np.float32)
            if i == 0
            else np.zeros((P, D), np.float32)
        )
        in_maps.append(
            {
                "qT": q2,
                "kT": k2,
                "vT": v2,
                "wq": np.ascontiguousarray(w_q[:, sl]).astype(bf16),
                "wk": np.ascontiguousarray(w_k[:, sl]).astype(bf16),
                "wv": np.ascontiguousarray(w_v[:, sl]).astype(bf16),
                "bq": np.ascontiguousarray(b_q[sl]).reshape(P, 1),
                "bk": np.ascontiguousarray(b_k[sl]).reshape(P, 1),
                "bv": np.ascontiguousarray(b_v[sl]).reshape(P, 1),
                "wo": np.ascontiguousarray(w_o[sl, :]).astype(bf16),
                "bo": bo_i,
            }
        )

    kwargs = {}
    if TRACE:
        tdir = "/tmp/bass_trace"
        os.makedirs(tdir, exist_ok=True)
        kwargs["tmpdir"] = tdir
    res = run_bass_kernel_spmd(nc, in_maps, list(range(NCORES)), trace=TRACE, **kwargs)
    LAST_RESULT = {
        "exec_time_ns": res.exec_time_ns,
        "trace_path": (res.instructions_and_trace or (None, None))[1],
    }
    acc = np.zeros((L, D), np.float64)
    for i in range(NCORES):
        acc += res.results[i]["out"].astype(np.float64)
    return acc.astype(np.float32).reshape(1, L, D)
